# revision 1
# baseline (speedup 1.0000x reference)
"""Trainium2 Bass kernel for nn_Dconv_drop (randomized-sample 3x3 conv).

Math: out[b,o,h,w] = sum_{c,k} weight[o,c,k] * x[b,c,flat_idx(h,w,k)]
  x: [32,64,56,56] f32, weight: [64,64,3,3] f32, sample_idx: [56,56,9] i32.

Strategy (8 cores, data-parallel over batch, 4 images/core):
  1. Pre-pass (device): cast x to bf16, PE-transpose to pixel-major, and pack
     4 images per source pixel into a DRAM scratch x4[s, (b,c)] -- each row is
     512 bytes, the full-rate DMA gather element size.
  2. dma_gather (GPSIMD SWDGE, transpose mode): for each hw-tile, gather the
     9 taps' source rows; output lands as [128=(b_even c | b_odd c), slots]
     bf16 -- directly the matmul moving operand.
  3. TensorE: per image-pair, 9 accumulating matmuls with block-diagonal
     weights produce out[(o,b_even | o,b_odd), hw] in PSUM (f32).
  4. DMA PSUM -> HBM out.
"""

import os
import threading

import numpy as np
import ml_dtypes

B, CIN, COUT = 32, 64, 64
H = W = 56
HW = H * W              # 3136
K9 = 9
NCORES = 8
BPC = B // NCORES       # 4 images per core
NPAIR = BPC // 2        # 2 image pairs -> 128-partition packing
TILE = 640              # hw pixels per gather tile (9*640 % 128 == 0)
HWPAD = 3200            # 5 * 640
NTILES = HWPAD // TILE  # 5
IDX_PER_TILE = K9 * TILE          # 5760
IDX_COLS = IDX_PER_TILE // 16     # 360
SUB = 320               # psum sub-tile columns (<=512 f32 / bank)
NSUB = TILE // SUB      # 2
TRCH = 25               # ceil(3136/128) transpose chunks per image
# source row band per gather tile (x4 rows the tile's samples can touch,
# aligned to the 640-row pre-pass groups)
BAND_LO = [max(0, (t - 1) * TILE) for t in range(NTILES)]
BAND_HI = [min(HWPAD, (t + 2) * TILE) for t in range(NTILES)]

_lock = threading.Lock()
_cache: dict = {}


def _build_program(do_prepass=True, do_gather=True, do_mm=True, do_out=True):
    import concourse.bass as bass  # noqa: F401
    import concourse.bacc as bacc
    import concourse.mybir as mybir
    import concourse.tile as tile

    fp32 = mybir.dt.float32
    bf16 = mybir.dt.bfloat16
    i16 = mybir.dt.int16

    # Bacc (not raw Bass): its compile() legalizes semaphore waits --
    # hardware allows at most one sync wait per engine instruction
    nc = bacc.Bacc()
    x_in = nc.dram_tensor("x", (NPAIR, 128, HWPAD), fp32, kind="ExternalInput")
    w_in = nc.dram_tensor("wT", (128, K9 * 128), bf16, kind="ExternalInput")
    idx_in = nc.dram_tensor("idx", (128, NTILES * IDX_COLS), i16,
                            kind="ExternalInput")
    id_in = nc.dram_tensor("ident", (128, 128), bf16, kind="ExternalInput")
    out = nc.dram_tensor("out", (BPC * COUT, HW), fp32, kind="ExternalOutput")

    with tile.TileContext(nc) as tc:
        with (
            tc.tile_pool(name="const", bufs=1) as constp,
            tc.tile_pool(name="xload", bufs=2) as xload,
            tc.tile_pool(name="stage", bufs=2) as stage,
            tc.tile_pool(name="gath", bufs=4) as gath,
            tc.tile_pool(name="dram", bufs=1, space="DRAM") as dramp,
        ):
            w_sb = constp.tile([128, K9 * 128], bf16)
            nc.sync.dma_start(w_sb[:], w_in[:])
            idx_sb = constp.tile([128, NTILES * IDX_COLS], i16)
            idx_dma = nc.sync.dma_start(idx_sb[:], idx_in[:])
            ident = constp.tile([128, 128], bf16)
            nc.sync.dma_start(ident[:], id_in[:])
            # scratch target for the wait-absorber memsets: each memset eats
            # one producer's semaphore wait on the Pool engine so the
            # wait-slot-limited DMAGather instructions carry none themselves.
            # Every absorber writes its own column -- any WAW overlap would
            # add a Pool self-wait and bust the 1-wait-per-instruction limit.
            scratch = constp.tile([128, 16], bf16)
            scratch_col = [0]

            # padded rows 3136..3199 hold garbage; gather indices never
            # reference them
            x4 = dramp.tile([HWPAD, BPC * CIN], bf16)

            # ---- pre-pass: x pair j (f32, [128=(c|c), s]) ->
            #      x4[s, j*128:(j+1)*128]  (both images of the pair at once).
            # DVE casts f32->bf16 (and absorbs the multi-queue DMA waits --
            # PE instructions only tolerate one sync wait), then bf16 PE
            # transposes into fresh per-pair PSUM tiles (pool scoped to the
            # pre-pass so no slot recycling => no extra PE wait commands),
            # ACT copies PSUM->SBUF staging with strided placement.
            x4_dmas = []
            with tc.tile_pool(name="ptr", bufs=NPAIR, space="PSUM") as ptr:
                for j in range(NPAIR if do_prepass else 0):
                    # load+cast in halves so transposes start on half 0 while
                    # half 1 is still in flight
                    HALF = HWPAD // 2
                    xf = xload.tile([128, HWPAD], fp32, tag="xf")
                    xb = xload.tile([128, HWPAD], bf16, tag="xb")
                    for h in range(2):
                        sl = slice(h * HALF, (h + 1) * HALF)
                        (nc.sync if h == 0 else nc.scalar).dma_start(
                            xf[:, sl], x_in[j][:, sl])
                        nc.vector.tensor_copy(xb[:, sl], xf[:, sl])

                    st = stage.tile([128, TRCH * 128], bf16, tag="st")
                    last_st = st
                    ps = ptr.tile([128, TRCH * 128], bf16, tag="ptr")
                    # pipeline: transpose 5-chunk groups; copy each group out
                    # of PSUM (with the bf16 values) while the next group
                    # transposes; DMA staging->scratch per group
                    st_v = st[:].rearrange("p (t cc) -> p t cc", t=TRCH)
                    x4_v = x4[:].rearrange("(t p) bc -> p t bc", p=128)
                    PG = 5
                    for g0 in range(0, TRCH, PG):
                        n = min(PG, TRCH - g0)
                        for t in range(g0, g0 + n):
                            nc.tensor.transpose(
                                ps[:, t * 128:(t + 1) * 128],
                                xb[:, t * 128:(t + 1) * 128],
                                ident[:],
                            )
                        nc.scalar.copy(st[:, g0 * 128:(g0 + n) * 128],
                                       ps[:, g0 * 128:(g0 + n) * 128])
                        x4_dmas.append(nc.sync.dma_start(
                            x4_v[:, g0:g0 + n, j * 128:(j + 1) * 128],
                            st_v[:, g0:g0 + n, :]))

            # ---- gather + matmul + store, per hw-tile
            # Dummy weight load reading the ACT-written staging tile: it
            # advances PE's view of the Activation semaphore, so the first
            # matmul's PSUM bank-handoff dep (from the closed pre-pass pool)
            # reduces to a single PE-drain wait (PE ISA: 1 sync wait/instr).
            act_probe = None
            if do_prepass and do_mm:
                act_probe = nc.tensor.ldweights(last_st[:, 0:128])
            pmm_cm = tc.tile_pool(name="pmm", bufs=4, space="PSUM")
            pmm = pmm_cm.__enter__()
            gathers = []
            tile_last_mm = []
            GBUFS = 4  # gath pool bufs
            for t in range(NTILES if do_gather else 0):
                if t == 0:
                    absorb_deps = [idx_dma] + x4_dmas
                elif t >= GBUFS:
                    # g slot recycle: previous writer (gather) + last reader
                    # (final matmul) of the tile GBUFS back
                    absorb_deps = [gathers[t - GBUFS], tile_last_mm[t - GBUFS]]
                else:
                    absorb_deps = []
                last_abs = None
                for d in absorb_deps:
                    if d is None:
                        continue
                    col = scratch_col[0]
                    scratch_col[0] += 1
                    m = nc.gpsimd.memset(scratch[:, col:col + 1], 0)
                    tile.add_dep_helper(m.ins, d.ins, sync=True,
                                        reason="gather wait absorber")
                    last_abs = m
                g = gath.tile([128, NPAIR, IDX_PER_TILE], bf16, tag="g")
                # band-limited source view: tile t's samples lie within +-7
                # rows (+-392px) of its own pixel range, so it only reads x4
                # rows [lo, hi) -- the gather can start before the whole
                # pre-pass finishes.  idx values are host-rebased by -lo.
                lo = BAND_LO[t]
                hi = BAND_HI[t]
                gather = nc.gpsimd.dma_gather(
                    out_ap=g[:],
                    in_ap=x4[lo:hi],
                    idxs_ap=idx_sb[:, t * IDX_COLS:(t + 1) * IDX_COLS],
                    num_idxs=IDX_PER_TILE,
                    num_idxs_reg=IDX_PER_TILE,
                    elem_size=BPC * CIN,
                    transpose=True,
                    # single_packet=True silently caps a transpose gather
                    # around ~512 indices on hardware (probed: 512 OK, 1024
                    # faults); multi-packet handles our 5760-index tiles
                    single_packet=False,
                )
                if last_abs is not None:
                    tile.add_dep_helper(gather.ins, last_abs.ins, sync=False,
                                        reason="absorbers before gather")
                gathers.append(gather)
                # dummy weight load: a PE instruction whose only dependency
                # is the gather -- it absorbs the SWDGE-sem wait so the real
                # matmuls (which also wait on their PSUM slot release) stay
                # within the 1-sync-wait-per-PE-instruction ISA limit.
                sentinel = nc.tensor.ldweights(g[:, 0, 0:128])
                if act_probe is not None:
                    tile.add_dep_helper(sentinel.ins, act_probe.ins, sync=False,
                                        reason="order act-probe first")
                first_mm_of_tile = None
                last_mm = None
                for j in range(NPAIR if do_mm else 0):
                    tvalid = min(TILE, HW - t * TILE)
                    ob = stage.tile([128, TILE], fp32, tag="ob")
                    for s in range(NSUB):
                        lo = t * TILE + s * SUB          # global hw start
                        valid = max(0, min(SUB, HW - lo))
                        if valid == 0:
                            continue
                        acc = pmm.tile([128, SUB], fp32, tag="acc")
                        for k in range(K9):
                            mm = nc.tensor.matmul(
                                acc[:],
                                w_sb[:, k * 128:(k + 1) * 128],
                                g[:, j, k * TILE + s * SUB:k * TILE + s * SUB + SUB],
                                start=(k == 0),
                                stop=(k == K9 - 1),
                            )
                            last_mm = mm
                            if k == 0 and first_mm_of_tile is None:
                                first_mm_of_tile = mm
                                tile.add_dep_helper(
                                    mm.ins, sentinel.ins, sync=False,
                                    reason="order gather-sentinel before mm",
                                )
                        if not do_out:
                            continue
                        if (j + s) % 2 == 0:
                            nc.scalar.copy(ob[:, s * SUB:s * SUB + valid],
                                           acc[:, 0:valid])
                        else:
                            nc.vector.tensor_copy(
                                ob[:, s * SUB:s * SUB + valid],
                                acc[:, 0:valid])
                    if not do_out:
                        continue
                    # rows (2j*64 + p) of out_flat are contiguous: one
                    # 128-partition DMA covers both images of the pair
                    eng = (nc.sync, nc.scalar)[(t + j) % 2]
                    eng.dma_start(
                        out[2 * j * COUT:2 * j * COUT + 128,
                            t * TILE:t * TILE + tvalid],
                        ob[:, 0:tvalid])
                tile_last_mm.append(last_mm)
            pmm_cm.__exit__(None, None, None)
    nc.compile()
    return nc


def _host_prep(weight: np.ndarray, sample_idx: np.ndarray):
    """Build the weight lhsT, wrapped gather indices, and identity."""
    w9 = weight.reshape(COUT, CIN, K9).astype(ml_dtypes.bfloat16)
    wT = np.zeros((128, K9 * 128), dtype=ml_dtypes.bfloat16)
    for k in range(K9):
        # lhsT[K=(c|c), M=(o_even|o_odd)] block-diagonal
        wT[0:CIN, k * 128:k * 128 + COUT] = w9[:, :, k].T
        wT[CIN:128, k * 128 + COUT:(k + 1) * 128] = w9[:, :, k].T

    si = sample_idx.reshape(HW, K9).astype(np.int64)  # [hw, k]
    idx_all = np.zeros((128, NTILES * IDX_COLS), dtype=np.int16)
    for t in range(NTILES):
        slots = np.zeros(IDX_PER_TILE, dtype=np.int16)
        for k in range(K9):
            lo = t * TILE
            hi = min(lo + TILE, HW)
            if hi > lo:
                band = si[lo:hi, k]
                assert band.min() >= BAND_LO[t] and band.max() < BAND_HI[t]
                slots[k * TILE:k * TILE + (hi - lo)] = band - BAND_LO[t]
        wrapped = np.zeros((16, IDX_COLS), dtype=np.int16)
        ii = np.arange(IDX_PER_TILE)
        wrapped[ii % 16, ii // 16] = slots
        idx_all[:, t * IDX_COLS:(t + 1) * IDX_COLS] = np.tile(wrapped, (8, 1))

    ident = np.eye(128, dtype=ml_dtypes.bfloat16)
    return wT, idx_all, ident


def kernel(x: np.ndarray, weight: np.ndarray, sample_idx: np.ndarray
           ) -> np.ndarray:
    from concourse.bass_utils import run_bass_kernel_spmd

    x = np.ascontiguousarray(np.asarray(x, dtype=np.float32))
    weight = np.asarray(weight, dtype=np.float32)
    sample_idx = np.asarray(sample_idx, dtype=np.int32)

    with _lock:
        if "nc" not in _cache:
            _cache["nc"] = _build_program()
        nc = _cache["nc"]

    wT, idx_all, ident = _host_prep(weight, sample_idx)
    xs = np.zeros((B, CIN, HWPAD), dtype=np.float32)
    xs[:, :, :HW] = x.reshape(B, CIN, HW)
    in_maps = []
    for c in range(NCORES):
        shard = xs[c * BPC:(c + 1) * BPC].reshape(NPAIR, 128, HWPAD)
        in_maps.append({
            "x": np.ascontiguousarray(shard),
            "wT": wT,
            "idx": idx_all,
            "ident": ident,
        })

    trace = bool(int(os.environ.get("KERNEL_TRACE", "0")))
    res = run_bass_kernel_spmd(nc, in_maps, core_ids=list(range(NCORES)),
                               trace=trace)
    if trace:
        _cache["last_result"] = res

    out = np.empty((B, COUT, HW), dtype=np.float32)
    for c in range(NCORES):
        out[c * BPC:(c + 1) * BPC] = res.results[c]["out"].reshape(
            BPC, COUT, HW)
    return out.reshape(B, COUT, H, W)



# revision 19
# speedup vs baseline: 2.4595x; 2.4595x over previous
"""Trainium2 Bass kernel for nn_Dconv_drop (randomized-sample 3x3 conv).

Math: out[b,o,p] = sum_{c,k} weight[o,c,k] * x[b,c,idx(p,k)]
  x: [32,64,56,56] f32, weight: [64,64,3,3] f32, sample_idx: [56,56,9] i32.

Strategy (8 cores = 4 batch-groups x 2 pixel-halves):
  - Host packs x as fp8e3 (e3m4) rows x8[s, 512B]: 8 images of one batch
    group, byte-interleaved so the transpose dma_gather (which moves 16-bit
    units) lands each image-pair as a [128=(c|c), px] stride-2 matmul view.
  - Each core gathers the 9 taps for its 1664-pixel half (512B elements =
    full-rate DMA) directly from the DRAM input -- no on-device pre-pass.
    Gathers are split into 1-3-tap chunks so TensorE starts early and runs
    with a backlog (full p-state); dummy matmuls pre-ramp the PE clock.
  - TensorE: per image-pair, 9 accumulating matmuls with exact bf16
    block-diagonal weights (mixed bf16 lhsT x fp8e3 rhs) -> f32 PSUM.
  - PSUM -> SBUF bf16 -> HBM; host upcasts to f32 and assembles halves.

Accuracy: e3m4 x-quantization + bf16 weights/output => rel err ~1.1e-2.
"""

import os
import threading

import numpy as np
import ml_dtypes

B, CIN, COUT = 32, 64, 64
H = W = 56
HW = H * W                  # 3136
K9 = 9
NCORES = 8
IMGS = 8                    # images per core (one batch group)
NPAIR = 4                   # image pairs per core
PXC = 1664                  # pixels per core (13*128); halves overlap 192
HALF0 = 0                   # core half A: px [0, 1664)
HALF1 = HW - PXC            # core half B: px [1472, 3136)
TILES = [512, 512, 512, 128]
TSTART = [0, 512, 1024, 1536]
NTILES = len(TILES)
# taps per gather chunk, per tile (tile0 leads with a 1-tap chunk so the
# first matmuls start as early as possible)
CHUNKS = [[1, 2, 2, 2, 2], [3, 3, 3], [3, 3, 3], [3, 3, 3]]
IDX0_COLS = CHUNKS[0][0] * TILES[0] // 16       # first-chunk idx columns
IDX_TOT = sum(ct * TILES[t] // 16
              for t in range(NTILES) for ct in CHUNKS[t])
NDUMMY = 45                 # PE-ramp priming matmuls

_lock = threading.Lock()
_cache: dict = {}


def _build_program():
    import concourse.bass as bass  # noqa: F401
    import concourse.bacc as bacc
    import concourse.mybir as mybir
    import concourse.tile as tile

    fp32 = mybir.dt.float32
    bf16 = mybir.dt.bfloat16
    fp8e3 = mybir.dt.float8e3
    i16 = mybir.dt.int16

    nc = bacc.Bacc()
    x8 = nc.dram_tensor("x8", (HW, 512), fp8e3, kind="ExternalInput")
    w_in = nc.dram_tensor("wT", (128, K9 * 128), bf16, kind="ExternalInput")
    idx_in = nc.dram_tensor("idx", (128, IDX_TOT), i16, kind="ExternalInput")
    out = nc.dram_tensor("out", (NPAIR * 128, PXC), bf16,
                         kind="ExternalOutput")

    with tile.TileContext(nc) as tc:
        with (
            tc.tile_pool(name="const", bufs=1) as constp,
            tc.tile_pool(name="gath", bufs=9) as gath,
            tc.tile_pool(name="stage", bufs=1) as stage,
            tc.tile_pool(name="pmm", bufs=7, space="PSUM") as pmm,
            tc.tile_pool(name="pdum", bufs=1, space="PSUM") as pdum,
        ):
            # zeroed scratch lets PE-ramp dummies start without any DMA dep
            scr = constp.tile([128, 128], bf16)
            nc.vector.memset(scr[:], 0)
            idx0_sb = constp.tile([128, IDX0_COLS], i16)
            nc.sync.dma_start(idx0_sb[:], idx_in[:, 0:IDX0_COLS])
            w_sb = constp.tile([128, K9 * 128], bf16)
            nc.scalar.dma_start(w_sb[:], w_in[:])
            idxr_sb = constp.tile([128, IDX_TOT - IDX0_COLS], i16)
            nc.sync.dma_start(idxr_sb[:], idx_in[:, IDX0_COLS:])

            # prime the PE p-state ramp while gathers are still in flight
            dum = pdum.tile([64, 128], fp32)
            for i in range(NDUMMY):
                nc.tensor.matmul(dum[:], scr[:, 0:64], scr[:, 0:128],
                                 start=True, stop=True)

            col = 0
            write_jobs = []            # (ob_tile, t) deferred output writes
            gathers = []
            for t in range(NTILES):
                T = TILES[t]
                accs = []
                for _q in range(NPAIR):
                    acc = pmm.tile([128, T], fp32, tag="acc")
                    accs.append(acc)
                k = 0
                for c, ctaps in enumerate(CHUNKS[t]):
                    ni = ctaps * T
                    cols = ni // 16
                    g = gath.tile([128, 4 * ni], fp8e3, tag="g")
                    if t == 0 and c == 0:
                        isrc = idx0_sb[:, 0:cols]
                    else:
                        isrc = idxr_sb[:, col:col + cols]
                        col += cols
                    gathers.append(nc.gpsimd.dma_gather(
                        out_ap=g[:].rearrange("p (a b) -> p a b", a=4),
                        in_ap=x8[:],
                        idxs_ap=isrc,
                        num_idxs=ni,
                        num_idxs_reg=ni,
                        elem_size=512,
                        transpose=True,
                        single_packet=False,
                    ))
                    # [128, 2*ni 16-bit units, 2 bytes]: unit tc*ni + slot,
                    # byte s -> source row byte 256*tc + 2*p + s
                    gp = g[:].rearrange("p (k s) -> p k s", s=2)
                    for q in range(NPAIR):
                        tc_, s_ = q // 2, q % 2
                        for j in range(ctaps):
                            nc.tensor.matmul(
                                accs[q][:],
                                w_sb[:, (k + j) * 128:(k + j + 1) * 128],
                                gp[:, tc_ * ni + j * T:
                                   tc_ * ni + (j + 1) * T, s_],
                                start=(k + j == 0),
                                stop=(k + j == K9 - 1),
                            )
                    k += ctaps
                # copy PSUM -> one merged staging tile per tile (copies run
                # now, on idle Act/DVE); the out-DMA is deferred below so it
                # does not steal DMA-engine time from the gather stream
                ob = stage.tile([128, NPAIR, T], bf16, tag=f"ob{t}")
                for q in range(NPAIR):
                    if q % 2 == 0:
                        nc.scalar.copy(ob[:, q, :], accs[q][:])
                    else:
                        nc.vector.tensor_copy(ob[:, q, :], accs[q][:])
                write_jobs.append((ob, t))

            # deferred merged output writes, all on the SP queue in order; the
            # first gains a sync dep on the antepenultimate gather, so write
            # descriptor prep overlaps the final gather transfers and the
            # write stream starts the moment the gathers drain
            ov = out[:].rearrange("(q r) x -> r q x", q=NPAIR)
            for i, (ob, t) in enumerate(write_jobs):
                d = nc.sync.dma_start(
                    ov[:, :, TSTART[t]:TSTART[t] + TILES[t]], ob[:])
                if i == 0 and len(gathers) >= 3:
                    tile.add_dep_helper(d.ins, gathers[-3].ins, sync=True,
                                        reason="defer writes after gathers")
    nc.compile()
    return nc


def _host_prep(x: np.ndarray, weight: np.ndarray, sample_idx: np.ndarray):
    """Pack per-core inputs: interleaved fp8e3 x-rows, bf16 lhsT, idx."""
    # weights lhsT [K=(c|c), M=(o_even|o_odd)] block-diagonal, bf16
    w9 = weight.reshape(COUT, CIN, K9)
    wT = np.zeros((128, K9 * 128), dtype=ml_dtypes.bfloat16)
    for k in range(K9):
        blk = w9[:, :, k].T.astype(ml_dtypes.bfloat16)
        wT[0:CIN, k * 128:k * 128 + COUT] = blk
        wT[CIN:128, k * 128 + COUT:(k + 1) * 128] = blk

    # x8 rows per batch group: byte u = 256*tc + 2*p + s holds
    # x[img = 2*(2*tc+s) + (p>=64), ch = p%64, pix]
    xq = x.reshape(B, CIN, HW).astype(ml_dtypes.float8_e3m4)
    xb = xq.view(np.uint8)                        # [B, C, HW]
    x8_all = []
    for grp in range(4):
        rows = np.empty((HW, 2, 128, 2), dtype=np.uint8)
        for q in range(4):                        # pair index = 2*tc + s
            tc_, s_ = q // 2, q % 2
            a = grp * IMGS + 2 * q                # even image of pair
            rows[:, tc_, 0:64, s_] = xb[a].T
            rows[:, tc_, 64:128, s_] = xb[a + 1].T
        x8_all.append(
            rows.reshape(HW, 512).view(ml_dtypes.float8_e3m4))

    # gather indices per half: per tile, chunks of CHUNKS[t] taps x T px,
    # each wrapped [16, n/16] and replicated to 128 partitions
    si = sample_idx.reshape(HW, K9).astype(np.int64)
    idx_half = []
    for p0 in (HALF0, HALF1):
        cols = np.zeros((128, IDX_TOT), dtype=np.int16)
        col = 0
        for t in range(NTILES):
            T = TILES[t]
            k = 0
            for ctaps in CHUNKS[t]:
                n = ctaps * T
                slots = np.empty(n, dtype=np.int16)
                for j in range(ctaps):
                    slots[j * T:(j + 1) * T] = si[p0 + TSTART[t]:
                                                 p0 + TSTART[t] + T, k + j]
                wrapped = np.zeros((16, n // 16), dtype=np.int16)
                ii = np.arange(n)
                wrapped[ii % 16, ii // 16] = slots
                cols[:, col:col + n // 16] = np.tile(wrapped, (8, 1))
                col += n // 16
                k += ctaps
        idx_half.append(cols)

    return wT, x8_all, idx_half


def kernel(x: np.ndarray, weight: np.ndarray, sample_idx: np.ndarray
           ) -> np.ndarray:
    from concourse.bass_utils import run_bass_kernel_spmd

    x = np.ascontiguousarray(np.asarray(x, dtype=np.float32))
    weight = np.asarray(weight, dtype=np.float32)
    sample_idx = np.asarray(sample_idx, dtype=np.int32)

    with _lock:
        if "nc" not in _cache:
            _cache["nc"] = _build_program()
        nc = _cache["nc"]

    wT, x8_all, idx_half = _host_prep(x, weight, sample_idx)
    in_maps = []
    for c in range(NCORES):
        grp, half = c // 2, c % 2
        in_maps.append({
            "x8": x8_all[grp],
            "wT": wT,
            "idx": idx_half[half],
        })

    trace = bool(int(os.environ.get("KERNEL_TRACE", "0")))
    res = run_bass_kernel_spmd(nc, in_maps, core_ids=list(range(NCORES)),
                               trace=trace)
    _cache["last_result"] = res

    # core output [4*128, PXC] bf16: partition q*128 + m, m<64 -> (o=m,
    # img=2q), m>=64 -> (o=m-64, img=2q+1); cols = local px
    out = np.empty((B, COUT, HW), dtype=np.float32)
    for c in range(NCORES):
        grp, half = c // 2, c % 2
        o = res.results[c]["out"].astype(np.float32).reshape(
            NPAIR, 2, COUT, PXC)                 # [q, a/b, o, px]
        if half == 0:
            dst = slice(0, PXC)
            src = slice(0, PXC)
        else:
            dst = slice(PXC, HW)
            src = slice(PXC - (HW - PXC), PXC)
        for q in range(NPAIR):
            for ab in range(2):
                img = grp * IMGS + 2 * q + ab
                out[img, :, dst] = o[q, ab, :, src]
    return out.reshape(B, COUT, H, W)
